# revision 24
# baseline (speedup 1.0000x reference)
"""EHR memory-network kernel for Trainium2 (8 NeuronCores, data-parallel over batch).

Reformulation of the reference scatter-scan:
  For patient b the scan applies, per event e (in time order), the affine update
      M[id_e] = M[id_e] * Af[e] + Bf[e]
  Since slot 0 is never touched (ids >= 1) and every touched slot starts from the
  same init_mem vector, the final row for node n is
      M[n] = init_mem * A_tot[n] + B_tot[n]
  with A_tot[n] = prod_{e: id_e=n} Af[e],  B_tot[n] = sum_{e: id_e=n} Bf[e]*SufA[e],
  SufA[e] = prod_{j>e, id_j=id_e} Af[j].  Af entries lie in (0,1] so products are
  exp(sum ln Af) and the id-grouped sums become matmuls against on-device compare
  matrices G[j,e] = (id_j == id_e) (strict-lower-triangle-masked for the suffix).
  Every event scatters its node's final row via indirect DMA; duplicate ids write
  identical values.

Host-side prep is index-only: valid-(t,mod)-group compaction (mask/valid_mod),
load-balanced patient->core assignment, gather/scatter index lists, per-slot id
offsets.  sigmoid is computed as (1+tanh(z/2))/2 folded into per-partition scale
vectors so the scalar engine only ever needs {tanh, exp} + {ln} table sets.
"""

import numpy as np
from contextlib import ExitStack

import concourse.bass as bass
import concourse.tile as tile
from concourse import bacc, mybir
from concourse import bass_utils
from concourse.bass import IndirectOffsetOnAxis

F32 = mybir.dt.float32
R32 = mybir.dt.float32r
I32 = mybir.dt.int32
AF = mybir.ActivationFunctionType
OP = mybir.AluOpType

# Problem shapes (hardcoded per contest contract).
B, T, MOD, D = 32, 64, 3, 4
WD, MEM, HID, DEMO = 256, 256, 512, 64
N_NODES = 4096
N_CORES = 8
BP = B // N_CORES              # patient slots per core = 4
NCH = 7                        # event chunks of 128 per core
S_C = NCH * 128                # events per core = 896 (224 (t,mod) groups)
X_ROWS = BP * T * MOD * D      # rows of per-core x (3072)
OUT_ROWS = BP * N_NODES        # 16384
PAD_ROWS = 128                 # scratch rows for padding-event scatters
NCOL = 2                       # leading chunks holding all multi-hit slots
NCE = NCOL * 128               # collision-region events = 256

_NC_CACHE = {}


def _build_nc():
    """Build the single-core Bass/Tile program (SPMD across the 8 cores)."""
    nc = bacc.Bacc("TRN2", target_bir_lowering=False, debug=False,
                   enable_asserts=False, num_devices=N_CORES)
    t = {}
    t["xT0"] = nc.dram_tensor("xT0", [128, S_C], F32, kind="ExternalInput").ap()
    t["xT1"] = nc.dram_tensor("xT1", [128, S_C], F32, kind="ExternalInput").ap()
    t["ids"] = nc.dram_tensor("ids", [S_C], I32, kind="ExternalInput").ap()
    t["validf"] = nc.dram_tensor("validf", [S_C], F32, kind="ExternalInput").ap()
    t["cvecs"] = nc.dram_tensor("cvecs", [128, 12], F32, kind="ExternalInput").ap()
    t["tri"] = nc.dram_tensor("tri", [128, 128], F32, kind="ExternalInput").ap()
    t["ident"] = nc.dram_tensor("ident", [128, 128], F32, kind="ExternalInput").ap()
    t["demo"] = nc.dram_tensor("demo", [BP, DEMO], F32, kind="ExternalInput").ap()
    t["W1"] = nc.dram_tensor("W1", [DEMO, HID], F32, kind="ExternalInput").ap()
    t["b1"] = nc.dram_tensor("b1", [HID], F32, kind="ExternalInput").ap()
    t["W2"] = nc.dram_tensor("W2", [HID, DEMO], F32, kind="ExternalInput").ap()
    t["b2"] = nc.dram_tensor("b2", [DEMO], F32, kind="ExternalInput").ap()
    t["W3"] = nc.dram_tensor("W3", [DEMO, MEM], F32, kind="ExternalInput").ap()
    t["b3"] = nc.dram_tensor("b3", [MEM], F32, kind="ExternalInput").ap()
    t["We"] = nc.dram_tensor("We", [WD, MEM], F32, kind="ExternalInput").ap()
    t["be"] = nc.dram_tensor("be", [MEM], F32, kind="ExternalInput").ap()
    t["Wa"] = nc.dram_tensor("Wa", [WD, MEM], F32, kind="ExternalInput").ap()
    t["ba"] = nc.dram_tensor("ba", [MEM], F32, kind="ExternalInput").ap()
    t["init_mem"] = nc.dram_tensor("init_mem", [MEM], F32, kind="ExternalInput").ap()
    t["out"] = nc.dram_tensor("out", [OUT_ROWS + PAD_ROWS, MEM], F32,
                              kind="ExternalOutput").ap()

    with tile.TileContext(nc) as tc:
        with ExitStack() as ctx:
            _emit(ctx, tc, **t)
    nc.compile()
    return nc


def _emit(ctx, tc, *, xT0, xT1, ids, validf, cvecs, tri, ident, demo,
          W1, b1, W2, b2, W3, b3, We, be, Wa, ba, init_mem, out):
    nc = tc.nc
    P = 128

    const = ctx.enter_context(tc.tile_pool(name="const", bufs=1))
    big = ctx.enter_context(tc.tile_pool(name="big", bufs=1))
    work = ctx.enter_context(tc.tile_pool(name="work", bufs=4))
    psum = ctx.enter_context(tc.tile_pool(name="psum", bufs=1, space="PSUM"))

    # ---------- x + weight loads first: they gate the EA matmul pipeline ----
    xT = [big.tile([P, S_C], R32, tag=f"xT{i}", name=f"xT{i}") for i in range(2)]
    H = S_C // 2
    nc.sync.dma_start(xT[0][:, 0:H], xT0[:, 0:H].bitcast(R32))
    nc.scalar.dma_start(xT[0][:, H:S_C], xT0[:, H:S_C].bitcast(R32))
    nc.gpsimd.dma_start(xT[1][:], xT1.bitcast(R32))
    WeWa_t = [const.tile([P, 2 * MEM], R32, tag=f"WeWa{i}", name=f"WeWa{i}")
              for i in range(2)]
    for i in range(2):
        nc.sync.dma_start(WeWa_t[i][:, 0:MEM], We[i * P:(i + 1) * P, :].bitcast(R32))
        nc.scalar.dma_start(WeWa_t[i][:, MEM:2 * MEM], Wa[i * P:(i + 1) * P, :].bitcast(R32))

    # ---------- constants / small loads ----------
    tri_t = const.tile([P, P], F32, tag="tri", name="tri")
    nc.sync.dma_start(tri_t[:], tri[:])
    id_t = const.tile([P, P], F32, tag="ident", name="ident")
    nc.sync.dma_start(id_t[:], ident[:])
    cv = const.tile([P, 12], F32, tag="cv", name="cv")  # 0-2: A_k; 3-5: S_k; 6-8: 1+A_k
    nc.sync.dma_start(cv[:], cvecs[:])
    ones_f = const.tile([1, P], F32, tag="ones_f", name="ones_f")
    nc.vector.memset(ones_f[:], 1.0)
    ones_row = const.tile([1, P], R32, tag="ones_row", name="ones_row")
    nc.vector.tensor_copy(ones_row[:], ones_f[:])
    beba_row = const.tile([1, 2 * MEM], R32, tag="beba_row", name="beba_row")
    nc.sync.dma_start(beba_row[:, 0:MEM], be[None, :].bitcast(R32))
    nc.sync.dma_start(beba_row[:, MEM:2 * MEM], ba[None, :].bitcast(R32))
    init_row = const.tile([1, MEM], F32, tag="init_row", name="init_row")
    nc.sync.dma_start(init_row[:], init_mem[None, :])
    init128 = const.tile([P, MEM], F32, tag="init128", name="init128")
    nc.gpsimd.partition_broadcast(init128[:], init_row[:])
    # Fill the whole out table with init_mem via one replicating DMA (stride-0
    # source): every scatter/demo write below overwrites its rows afterwards.
    nc.sync.dma_start(out[None, :, :],
                      init_row[0:1, None, :].to_broadcast([1, OUT_ROWS + PAD_ROWS, MEM]))

    # gather/scatter indices + validity, [128, NCH] (column c = chunk c)
    ids_t = const.tile([P, NCH], I32, tag="ids", name="ids")
    nc.sync.dma_start(ids_t[:], ids.rearrange("(c p) -> p c", p=P))
    val_t = const.tile([P, NCH], F32, tag="val", name="val")
    nc.sync.dma_start(val_t[:], validf.rearrange("(c p) -> p c", p=P))
    idsf_t = const.tile([P, NCH], F32, tag="idsf", name="idsf")
    nc.vector.tensor_copy(idsf_t[:], ids_t[:])
    ids_row = const.tile([1, NCE], F32, tag="ids_row", name="ids_row")
    idsr_i = const.tile([1, NCE], I32, tag="ids_row_i", name="ids_row_i")
    nc.sync.dma_start(idsr_i[:], ids[None, 0:NCE])
    nc.vector.tensor_copy(ids_row[:], idsr_i[:])
    ids_row128 = const.tile([P, NCE], F32, tag="ids_row128", name="ids_row128")
    nc.gpsimd.partition_broadcast(ids_row128[:], ids_row[:])

    # Per-partition scale constants (tanh half-angle: sigmoid(z)=(1+tanh(z/2))/2):
    # Mk_k = 1 - ck*sig = (1 - ck/2) - (ck/2) tanh(z/2). Pads compute garbage
    # rows scattered to scratch, so no validity masking is needed and the same
    # [P,1] columns serve every chunk: cv cols 0-2 = A_k, 3-5 = S_k, 6-8 = 1+A_k.

    # ---------- demographics residual block (tiny, W-stationary) ----------
    demoT = const.tile([DEMO, BP], F32, tag="demoT", name="demoT")
    nc.sync.dma_start(demoT[:], demo.rearrange("b d -> d b"))
    W1_t = [const.tile([DEMO, P], F32, tag=f"W1{i}", name=f"W1{i}") for i in range(4)]
    b1_t = const.tile([P, 4], F32, tag="b1c", name="b1c")
    nc.sync.dma_start(b1_t[:], b1.rearrange("(i p) -> p i", p=P))
    W2_t = [const.tile([P, DEMO], F32, tag=f"W2{i}", name=f"W2{i}") for i in range(4)]
    W3_t = [const.tile([DEMO, P], F32, tag=f"W3{i}", name=f"W3{i}") for i in range(2)]
    b2_t = const.tile([DEMO, 1], F32, tag="b2c", name="b2c")
    nc.sync.dma_start(b2_t[:], b2[:, None])
    b3_t = const.tile([P, 2], F32, tag="b3c", name="b3c")
    nc.sync.dma_start(b3_t[:], b3.rearrange("(i p) -> p i", p=P))
    for i in range(4):
        nc.sync.dma_start(W1_t[i][:], W1[:, i * P:(i + 1) * P])
        nc.sync.dma_start(W2_t[i][:], W2[i * P:(i + 1) * P, :])
    for i in range(2):
        nc.sync.dma_start(W3_t[i][:], W3[:, i * P:(i + 1) * P])

    hT = [const.tile([P, BP], F32, tag=f"hT{i}", name=f"hT{i}") for i in range(4)]
    for i in range(4):
        ps = psum.tile([P, BP], F32, tag="tp", bufs=2, name="demo_ps")
        nc.tensor.matmul(ps[:], lhsT=W1_t[i][:], rhs=demoT[:], start=True, stop=True)
        nc.scalar.activation(hT[i][:], ps[:], AF.Relu, bias=b1_t[:, i:i + 1], scale=1.0)
    ps_y = psum.tile([DEMO, BP], F32, tag="psSuf", bufs=2, name="demo_y")
    for i in range(4):
        nc.tensor.matmul(ps_y[:], lhsT=W2_t[i][:], rhs=hT[i][:],
                         start=(i == 0), stop=(i == 3))
    yT = const.tile([DEMO, BP], F32, tag="yT", name="yT")
    nc.scalar.activation(yT[:], ps_y[:], AF.Identity, bias=b2_t[:, 0:1], scale=1.0)
    nc.vector.tensor_add(yT[:], yT[:], demoT[:])
    deT = [const.tile([P, BP], F32, tag=f"deT{i}", name=f"deT{i}") for i in range(2)]
    for i in range(2):
        ps = psum.tile([P, BP], F32, tag="tp", bufs=2, name="demo_ps2")
        nc.tensor.matmul(ps[:], lhsT=W3_t[i][:], rhs=yT[:], start=True, stop=True)
        nc.scalar.activation(deT[i][:], ps[:], AF.Identity, bias=b3_t[:, i:i + 1], scale=1.0)
    de_t = []
    for i in range(2):
        ps = psum.tile([BP, P], F32, tag="tp", bufs=2, name="demo_tp")
        nc.tensor.transpose(ps[:], deT[i][:], id_t[:])
        de = const.tile([BP, P], F32, tag=f"de{i}", name=f"de{i}")
        nc.vector.tensor_copy(de[:], ps[:])
        de_t.append(de)
    for i in range(2):
        nc.sync.dma_start(
            out[0:OUT_ROWS, :].rearrange("(b n) m -> b n m", b=BP)[:, 0, i * P:(i + 1) * P],
            de_t[i][:])

    # ---------- main pipeline over the 7 event chunks ----------
    lnAf = big.tile([P, NCOL * MEM], R32, tag="lnAf", name="lnAf")
    contrib = big.tile([P, NCOL * MEM], R32, tag="contrib", name="contrib")
    eAll_t = big.tile([P, NCOL * MEM], F32, tag="eAll_t", name="eAll_t")
    Al_t = big.tile([P, NCH * MEM], F32, tag="Al_t", name="Al_t")
    Bf = big.tile([P, NCH * MEM], F32, tag="Bf", name="Bf")
    G = big.tile([P, NCOL * NCE], R32, tag="G", name="G")
    Gd = big.tile([P, NCOL * P], R32, tag="Gd", name="Gd")
    Gd2 = big.tile([P, NCOL * P], R32, tag="Gd2", name="Gd2")

    def cc(c, w):
        return slice(c * w, (c + 1) * w)

    def gblk(J, E_):
        return G[:, J * NCE + E_ * P: J * NCE + E_ * P + P]

    def emit_collision_logexp():
        # ln + suffix/total log-composition for the collision region.
        # SufLog (j>e) accumulates in psS; after exp'ing it, the prefix (j<=e)
        # keeps accumulating into the same bank so it becomes AllLog for free.
        nc.scalar.activation(lnAf[:], Al_t[:, 0:NCOL * MEM], AF.Ln)
        for E_ in range(NCOL):
            ps = psum.tile([P, MEM], F32, tag=("psSuf" if E_ % 2 else "tp"),
                           bufs=2, name="psSuf")
            js = list(range(E_, NCOL))
            for n_, J in enumerate(js):
                lhsT = Gd[:, cc(E_, P)] if J == E_ else gblk(J, E_)
                nc.tensor.matmul(ps[:], lhsT=lhsT, rhs=lnAf[:, cc(J, MEM)],
                                 start=(n_ == 0), stop=(n_ == len(js) - 1))
            eSuf = work.tile([P, MEM], F32, tag="eSuf", name="eSuf")
            nc.scalar.activation(eSuf[:], ps[:], AF.Exp)
            nc.gpsimd.tensor_tensor(contrib[:, cc(E_, MEM)], Bf[:, cc(E_, MEM)],
                                    eSuf[:], op=OP.mult)
            for J in range(0, E_ + 1):
                lhsT = Gd2[:, cc(E_, P)] if J == E_ else gblk(J, E_)
                nc.tensor.matmul(ps[:], lhsT=lhsT, rhs=lnAf[:, cc(J, MEM)],
                                 start=False, stop=(J == E_), skip_group_check=True)
            nc.scalar.activation(eAll_t[:, cc(E_, MEM)], ps[:], AF.Exp)

    def emit_collision_rows():
        for E_ in range(NCOL):
            psB = psum.tile([P, MEM], F32, tag="psB", bufs=2, name="psB")
            for J in range(NCOL):
                nc.tensor.matmul(psB[:], lhsT=gblk(J, E_),
                                 rhs=contrib[:, cc(J, MEM)],
                                 start=(J == 0), stop=(J == NCOL - 1))
            row = work.tile([P, MEM], F32, tag="row", name="row")
            nc.gpsimd.tensor_tensor(row[:], eAll_t[:, cc(E_, MEM)], init128[:],
                                    op=OP.mult)
            nc.vector.tensor_add(row[:], row[:], psB[:])
            nc.gpsimd.indirect_dma_start(
                out=out[:], out_offset=IndirectOffsetOnAxis(ap=ids_t[:, E_:E_ + 1], axis=0),
                in_=row[:], in_offset=None)

    th_t = big.tile([P, NCH * MEM], F32, tag="th_t", name="th_t")
    A_t = big.tile([P, NCH * MEM], F32, tag="A_t", name="A_t")
    init2 = const.tile([P, 2 * MEM], F32, tag="init2", name="init2")
    nc.vector.tensor_copy(init2[:, 0:MEM], init128[:])
    nc.vector.tensor_copy(init2[:, MEM:2 * MEM], init128[:])

    for c in range(NCH):
        # E/A gate matmuls (event-major out), fused into one [128,512] psum
        psEA = psum.tile([P, 2 * MEM], F32, tag="psEA", bufs=2, name="psEA")
        nc.tensor.matmul(psEA[:], lhsT=ones_row[:], rhs=beba_row[:],
                         start=True, stop=False)
        for i in range(2):
            nc.tensor.matmul(psEA[:], lhsT=xT[i][:, cc(c, P)], rhs=WeWa_t[i][:],
                             start=False, stop=(i == 1))
        nc.scalar.activation(th_t[:, cc(c, MEM)], psEA[:, 0:MEM], AF.Tanh, scale=0.5)
        nc.scalar.activation(A_t[:, cc(c, MEM)], psEA[:, MEM:2 * MEM], AF.Tanh)

    # D-level chain composition over chunk PAIRS (validity-free constants make
    # the scale columns chunk-independent).  Phase 1 launches every pair's
    # MkMs builds + partition-shift DMAs so the shift latency overlaps; phase 2
    # consumes them with the multiply/accumulate chains.
    PAIRS = [(0, 2), (2, 2), (4, 2), (6, 1)]
    MkMs_p, sh_p = {}, {}
    for pi, (c0, w) in enumerate(PAIRS):
        W = w * MEM
        sl = slice(c0 * MEM, c0 * MEM + W)
        th = th_t[:, sl]
        A = A_t[:, sl]
        MkMs = [work.tile([P, 2 * W], F32, tag=f"MkMs{k}", name=f"MkMs{k}")
                for k in range(3)]
        sh = [work.tile([P, 2 * W], F32, tag=f"sh{k}", name=f"sh{k}")
              for k in range(3)]
        MkMs_p[pi], sh_p[pi] = MkMs, sh
        for k in range(3):
            nc.vector.tensor_scalar(MkMs[k][:, 0:W], th, cv[:, k:k + 1],
                                    cv[:, 6 + k:7 + k], op0=OP.mult, op1=OP.add)
            nc.vector.tensor_scalar_mul(MkMs[k][:, W:2 * W], A, cv[:, 3 + k:4 + k])
            eng = (nc.sync, nc.scalar, nc.gpsimd)[(pi * 3 + k) % 3]
            eng.dma_start(sh[k][0:P - 1 - k, :], MkMs[k][1 + k:P, :])
        nc.vector.tensor_scalar(Al_t[:, sl], th, -0.5, 0.5, op0=OP.mult, op1=OP.add)
        nc.vector.tensor_copy(Bf[:, sl], A)
        if c0 == 0:
            for c in range(NCOL):
                # compare matrix rows for this j-chunk (ids are slot-unique)
                nc.vector.tensor_tensor(G[:, cc(c, NCE)],
                                        idsf_t[:, c:c + 1].to_broadcast([P, NCE]),
                                        ids_row128[:], op=OP.is_equal)
                nc.vector.tensor_mul(Gd[:, cc(c, P)],
                                     G[:, c * NCE + c * P: c * NCE + c * P + P], tri_t[:])
                nc.vector.tensor_tensor(Gd2[:, cc(c, P)],
                                        G[:, c * NCE + c * P: c * NCE + c * P + P],
                                        Gd[:, cc(c, P)], op=OP.subtract)
    for pi, (c0, w) in enumerate(PAIRS):
        W = w * MEM
        sl = slice(c0 * MEM, c0 * MEM + W)
        sh = sh_p[pi]
        Al = Al_t[:, sl]
        Bc = Bf[:, sl]
        # A chain (vector) and B chain (gpsimd)
        nc.vector.tensor_mul(Al[0:P - 1], Al[0:P - 1], sh[0][0:P - 1, 0:W])
        nc.vector.tensor_mul(Al[0:P - 2], Al[0:P - 2], sh[1][0:P - 2, 0:W])
        nc.vector.tensor_mul(Al[0:P - 3], Al[0:P - 3], sh[2][0:P - 3, 0:W])
        nc.gpsimd.tensor_tensor(Bc[0:P - 1], Bc[0:P - 1], sh[0][0:P - 1, 0:W], op=OP.mult)
        nc.gpsimd.tensor_tensor(Bc[0:P - 1], Bc[0:P - 1], sh[0][0:P - 1, W:2 * W], op=OP.add)
        nc.gpsimd.tensor_tensor(Bc[0:P - 2], Bc[0:P - 2], sh[1][0:P - 2, 0:W], op=OP.mult)
        nc.gpsimd.tensor_tensor(Bc[0:P - 2], Bc[0:P - 2], sh[1][0:P - 2, W:2 * W], op=OP.add)
        nc.gpsimd.tensor_tensor(Bc[0:P - 3], Bc[0:P - 3], sh[2][0:P - 3, 0:W], op=OP.mult)
        nc.gpsimd.tensor_tensor(Bc[0:P - 3], Bc[0:P - 3], sh[2][0:P - 3, W:2 * W], op=OP.add)
        if c0 == 0:
            nc.vector.tensor_scalar_max(Al, Al, 1e-30)
            emit_collision_logexp()
        else:
            # single-hit chunks: row = init*Al + Bf, scattered immediately
            rowd = work.tile([P, W], F32, tag="rowd", name="rowd")
            nc.gpsimd.tensor_tensor(rowd[:], Al, init2[:, 0:W], op=OP.mult)
            nc.vector.tensor_add(rowd[:], rowd[:], Bc)
            for j in range(w):
                nc.gpsimd.indirect_dma_start(
                    out=out[:],
                    out_offset=IndirectOffsetOnAxis(ap=ids_t[:, c0 + j:c0 + j + 1], axis=0),
                    in_=rowd[:, j * MEM:(j + 1) * MEM], in_offset=None)
        if pi == 2:
            emit_collision_rows()


def _assign_patients(gvalid):
    """Balanced 4-patients-per-core assignment by valid-group count (LPT)."""
    counts = gvalid.reshape(B, -1).sum(1)
    order = np.argsort(-counts, kind="stable")
    loads = [0] * N_CORES
    members = [[] for _ in range(N_CORES)]
    for p in order:
        c = min((c for c in range(N_CORES) if len(members[c]) < BP),
                key=lambda c: loads[c])
        members[c].append(int(p))
        loads[c] += int(counts[p])
    assert max(loads) * D <= S_C, f"core load {max(loads)} groups > {S_C // D}"
    return members


def _host_prep(inputs):
    """Index-only host prep: compaction, balancing, index tensors."""
    x = np.ascontiguousarray(np.asarray(inputs["input"], np.float32)).reshape(B, T * MOD * D, WD)
    mask = np.asarray(inputs["mask"])
    valid_mod = np.asarray(inputs["valid_mod"])
    node_ids = np.asarray(inputs["node_ids"])
    demo = np.ascontiguousarray(np.asarray(inputs["demo"], np.float32))

    dpat = np.arange(128) % 4
    cvecs = np.zeros((128, 12), np.float32)
    for k in (1, 2, 3):
        m = (dpat >= k).astype(np.float32)
        cvecs[:, k - 1] = -(2.0 ** -k) / 2.0 * m     # A_k: -ck/2 (tanh half-angle)
        cvecs[:, 2 + k] = (2.0 ** -k) * m            # S_k: ck
        cvecs[:, 5 + k] = 1.0 + cvecs[:, k - 1]      # B_k: 1 + A_k
    tri = np.tril(np.ones((128, 128), np.float32), -1)
    ident = np.eye(128, dtype=np.float32)

    gvalid = (mask[:, :, None] > 0) & (valid_mod > 0)   # [B, T, MOD]
    members = _assign_patients(gvalid)

    weights = {k: np.asarray(inputs[k], np.float32)
               for k in ("W1", "b1", "W2", "b2", "W3", "b3", "We", "be", "Wa", "ba",
                         "init_mem")}
    in_maps = []
    for core in range(N_CORES):
        pats = members[core]
        xg = np.zeros((S_C,), np.int32)
        idsv = np.empty((S_C,), np.int32)
        idsv[:] = OUT_ROWS + (np.arange(S_C) % PAD_ROWS)  # pads -> scratch rows
        vf = np.zeros((S_C,), np.float32)
        # groups containing any multi-hit-slot event go first (chunks 0..NCOL-1,
        # the only region the G composition covers); per patient in time order.
        col_list, norm_list = [], []
        for slot, b in enumerate(pats):
            tms = np.nonzero(gvalid[b].reshape(T * MOD))[0]
            idsm = node_ids[b].reshape(T * MOD, D)
            uniq, cnt = np.unique(idsm[tms].reshape(-1), return_counts=True)
            multi = set(uniq[cnt >= 2].tolist())
            for tm in tms:
                dst = col_list if any(int(v) in multi for v in idsm[tm]) else norm_list
                dst.append((slot, int(tm)))
        assert len(col_list) * D <= NCE, \
            f"core {core}: {len(col_list)} collision groups > {NCE // D}"
        e = 0
        for slot, tm in col_list + norm_list:
            b = pats[slot]
            for d in range(D):
                xg[e] = slot * (T * MOD * D) + tm * D + d
                idsv[e] = slot * N_NODES + int(
                    node_ids[b, tm // MOD, tm % MOD, d])
                vf[e] = 1.0
                e += 1
        xe = x[pats].reshape(X_ROWS, WD)[xg].T     # [WD, S_C]
        in_maps.append({
            "xT0": np.ascontiguousarray(xe[0:128]),
            "xT1": np.ascontiguousarray(xe[128:256]),
            "ids": idsv, "validf": vf,
            "cvecs": cvecs, "tri": tri, "ident": ident,
            "demo": np.ascontiguousarray(demo[pats]),
            **weights,
        })
    return in_maps, members


def get_nc():
    if "nc" not in _NC_CACHE:
        _NC_CACHE["nc"] = _build_nc()
    return _NC_CACHE["nc"]


def kernel(**inputs) -> np.ndarray:
    nc = get_nc()
    in_maps, members = _host_prep(inputs)
    res = bass_utils.run_bass_kernel_spmd(nc, in_maps, core_ids=list(range(N_CORES)))
    out = np.empty((B, N_NODES, MEM), np.float32)
    for core in range(N_CORES):
        block = res.results[core]["out"][:OUT_ROWS].reshape(BP, N_NODES, MEM)
        for slot, b in enumerate(members[core]):
            out[b] = block[slot]
    return out


if __name__ == "__main__":
    ref = {}
    exec(open("/root/problem/reference.py").read(), ref)
    inputs = {k: np.asarray(v) for k, v in ref["setup_inputs"]().items()}
    got = kernel(**inputs)
    want = np.asarray(ref["reference"](**inputs))
    err = np.abs(got - want).max() / np.abs(want).max()
    print("rel err:", err)



# revision 25
# speedup vs baseline: 1.0659x; 1.0659x over previous
"""EHR memory-network kernel for Trainium2 (8 NeuronCores, data-parallel over batch).

Reformulation of the reference scatter-scan:
  For patient b the scan applies, per event e (in time order), the affine update
      M[id_e] = M[id_e] * Af[e] + Bf[e]
  Since slot 0 is never touched (ids >= 1) and every touched slot starts from the
  same init_mem vector, the final row for node n is
      M[n] = init_mem * A_tot[n] + B_tot[n]
  with A_tot[n] = prod_{e: id_e=n} Af[e],  B_tot[n] = sum_{e: id_e=n} Bf[e]*SufA[e],
  SufA[e] = prod_{j>e, id_j=id_e} Af[j].  Af entries lie in (0,1] so products are
  exp(sum ln Af) and the id-grouped sums become matmuls against on-device compare
  matrices G[j,e] = (id_j == id_e) (strict-lower-triangle-masked for the suffix).
  Every event scatters its node's final row via indirect DMA; duplicate ids write
  identical values.

Host-side prep is index-only: valid-(t,mod)-group compaction (mask/valid_mod),
load-balanced patient->core assignment, gather/scatter index lists, per-slot id
offsets.  sigmoid is computed as (1+tanh(z/2))/2 folded into per-partition scale
vectors so the scalar engine only ever needs {tanh, exp} + {ln} table sets.
"""

import numpy as np
from contextlib import ExitStack

import concourse.bass as bass
import concourse.tile as tile
from concourse import bacc, mybir
from concourse import bass_utils
from concourse.bass import IndirectOffsetOnAxis

F32 = mybir.dt.float32
R32 = mybir.dt.float32r
BF16 = mybir.dt.bfloat16
I32 = mybir.dt.int32
AF = mybir.ActivationFunctionType
OP = mybir.AluOpType

# Problem shapes (hardcoded per contest contract).
B, T, MOD, D = 32, 64, 3, 4
WD, MEM, HID, DEMO = 256, 256, 512, 64
N_NODES = 4096
N_CORES = 8
BP = B // N_CORES              # patient slots per core = 4
NCH = 7                        # event chunks of 128 per core
S_C = NCH * 128                # events per core = 896 (224 (t,mod) groups)
X_ROWS = BP * T * MOD * D      # rows of per-core x (3072)
OUT_ROWS = BP * N_NODES        # 16384
PAD_ROWS = 128                 # scratch rows for padding-event scatters
NCOL = 2                       # leading chunks holding all multi-hit slots
NCE = NCOL * 128               # collision-region events = 256

_NC_CACHE = {}


def _build_nc():
    """Build the single-core Bass/Tile program (SPMD across the 8 cores)."""
    nc = bacc.Bacc("TRN2", target_bir_lowering=False, debug=False,
                   enable_asserts=False, num_devices=N_CORES)
    t = {}
    t["xT0"] = nc.dram_tensor("xT0", [128, S_C], F32, kind="ExternalInput").ap()
    t["xT1"] = nc.dram_tensor("xT1", [128, S_C], F32, kind="ExternalInput").ap()
    t["ids"] = nc.dram_tensor("ids", [S_C], I32, kind="ExternalInput").ap()
    t["validf"] = nc.dram_tensor("validf", [S_C], F32, kind="ExternalInput").ap()
    t["cvecs"] = nc.dram_tensor("cvecs", [128, 12], F32, kind="ExternalInput").ap()
    t["tri"] = nc.dram_tensor("tri", [128, 128], F32, kind="ExternalInput").ap()
    t["ident"] = nc.dram_tensor("ident", [128, 128], F32, kind="ExternalInput").ap()
    t["demo"] = nc.dram_tensor("demo", [BP, DEMO], F32, kind="ExternalInput").ap()
    t["W1"] = nc.dram_tensor("W1", [DEMO, HID], F32, kind="ExternalInput").ap()
    t["b1"] = nc.dram_tensor("b1", [HID], F32, kind="ExternalInput").ap()
    t["W2"] = nc.dram_tensor("W2", [HID, DEMO], F32, kind="ExternalInput").ap()
    t["b2"] = nc.dram_tensor("b2", [DEMO], F32, kind="ExternalInput").ap()
    t["W3"] = nc.dram_tensor("W3", [DEMO, MEM], F32, kind="ExternalInput").ap()
    t["b3"] = nc.dram_tensor("b3", [MEM], F32, kind="ExternalInput").ap()
    t["We"] = nc.dram_tensor("We", [WD, MEM], F32, kind="ExternalInput").ap()
    t["be"] = nc.dram_tensor("be", [MEM], F32, kind="ExternalInput").ap()
    t["Wa"] = nc.dram_tensor("Wa", [WD, MEM], F32, kind="ExternalInput").ap()
    t["ba"] = nc.dram_tensor("ba", [MEM], F32, kind="ExternalInput").ap()
    t["init_mem"] = nc.dram_tensor("init_mem", [MEM], F32, kind="ExternalInput").ap()
    t["out"] = nc.dram_tensor("out", [OUT_ROWS + PAD_ROWS, MEM], F32,
                              kind="ExternalOutput").ap()

    with tile.TileContext(nc) as tc:
        with ExitStack() as ctx:
            _emit(ctx, tc, **t)
    nc.compile()
    return nc


def _emit(ctx, tc, *, xT0, xT1, ids, validf, cvecs, tri, ident, demo,
          W1, b1, W2, b2, W3, b3, We, be, Wa, ba, init_mem, out):
    nc = tc.nc
    P = 128

    const = ctx.enter_context(tc.tile_pool(name="const", bufs=1))
    big = ctx.enter_context(tc.tile_pool(name="big", bufs=1))
    work = ctx.enter_context(tc.tile_pool(name="work", bufs=4))
    psum = ctx.enter_context(tc.tile_pool(name="psum", bufs=1, space="PSUM"))

    # ---------- x + weight loads first: they gate the EA matmul pipeline ----
    xT = [big.tile([P, S_C], R32, tag=f"xT{i}", name=f"xT{i}") for i in range(2)]
    H = S_C // 2
    nc.sync.dma_start(xT[0][:, 0:H], xT0[:, 0:H].bitcast(R32))
    nc.scalar.dma_start(xT[0][:, H:S_C], xT0[:, H:S_C].bitcast(R32))
    nc.gpsimd.dma_start(xT[1][:], xT1.bitcast(R32))
    WeWa_t = [const.tile([P, 2 * MEM], R32, tag=f"WeWa{i}", name=f"WeWa{i}")
              for i in range(2)]
    for i in range(2):
        nc.sync.dma_start(WeWa_t[i][:, 0:MEM], We[i * P:(i + 1) * P, :].bitcast(R32))
        nc.scalar.dma_start(WeWa_t[i][:, MEM:2 * MEM], Wa[i * P:(i + 1) * P, :].bitcast(R32))

    # ---------- constants / small loads ----------
    tri_t = const.tile([P, P], F32, tag="tri", name="tri")
    nc.sync.dma_start(tri_t[:], tri[:])
    id_t = const.tile([P, P], F32, tag="ident", name="ident")
    nc.sync.dma_start(id_t[:], ident[:])
    cv = const.tile([P, 12], F32, tag="cv", name="cv")  # 0-2: A_k; 3-5: S_k; 6-8: 1+A_k
    nc.sync.dma_start(cv[:], cvecs[:])
    ones_f = const.tile([1, P], F32, tag="ones_f", name="ones_f")
    nc.vector.memset(ones_f[:], 1.0)
    ones_row = const.tile([1, P], R32, tag="ones_row", name="ones_row")
    nc.vector.tensor_copy(ones_row[:], ones_f[:])
    beba_row = const.tile([1, 2 * MEM], R32, tag="beba_row", name="beba_row")
    nc.sync.dma_start(beba_row[:, 0:MEM], be[None, :].bitcast(R32))
    nc.sync.dma_start(beba_row[:, MEM:2 * MEM], ba[None, :].bitcast(R32))
    init_row = const.tile([1, MEM], F32, tag="init_row", name="init_row")
    nc.sync.dma_start(init_row[:], init_mem[None, :])
    init128 = const.tile([P, MEM], F32, tag="init128", name="init128")
    nc.gpsimd.partition_broadcast(init128[:], init_row[:])
    # Fill the whole out table with init_mem via one replicating DMA (stride-0
    # source): every scatter/demo write below overwrites its rows afterwards.
    nc.sync.dma_start(out[None, :, :],
                      init_row[0:1, None, :].to_broadcast([1, OUT_ROWS + PAD_ROWS, MEM]))

    # gather/scatter indices + validity, [128, NCH] (column c = chunk c)
    ids_t = const.tile([P, NCH], I32, tag="ids", name="ids")
    nc.sync.dma_start(ids_t[:], ids.rearrange("(c p) -> p c", p=P))
    val_t = const.tile([P, NCH], F32, tag="val", name="val")
    nc.sync.dma_start(val_t[:], validf.rearrange("(c p) -> p c", p=P))
    idsf_t = const.tile([P, NCH], F32, tag="idsf", name="idsf")
    nc.vector.tensor_copy(idsf_t[:], ids_t[:])
    ids_row = const.tile([1, NCE], F32, tag="ids_row", name="ids_row")
    idsr_i = const.tile([1, NCE], I32, tag="ids_row_i", name="ids_row_i")
    nc.sync.dma_start(idsr_i[:], ids[None, 0:NCE])
    nc.vector.tensor_copy(ids_row[:], idsr_i[:])
    ids_row128 = const.tile([P, NCE], F32, tag="ids_row128", name="ids_row128")
    nc.gpsimd.partition_broadcast(ids_row128[:], ids_row[:])

    # Per-partition scale constants (tanh half-angle: sigmoid(z)=(1+tanh(z/2))/2):
    # Mk_k = 1 - ck*sig = (1 - ck/2) - (ck/2) tanh(z/2). Pads compute garbage
    # rows scattered to scratch, so no validity masking is needed and the same
    # [P,1] columns serve every chunk: cv cols 0-2 = A_k, 3-5 = S_k, 6-8 = 1+A_k.

    # ---------- demographics residual block (tiny, W-stationary) ----------
    demoT = const.tile([DEMO, BP], F32, tag="demoT", name="demoT")
    nc.sync.dma_start(demoT[:], demo.rearrange("b d -> d b"))
    W1_t = [const.tile([DEMO, P], F32, tag=f"W1{i}", name=f"W1{i}") for i in range(4)]
    b1_t = const.tile([P, 4], F32, tag="b1c", name="b1c")
    nc.sync.dma_start(b1_t[:], b1.rearrange("(i p) -> p i", p=P))
    W2_t = [const.tile([P, DEMO], F32, tag=f"W2{i}", name=f"W2{i}") for i in range(4)]
    W3_t = [const.tile([DEMO, P], F32, tag=f"W3{i}", name=f"W3{i}") for i in range(2)]
    b2_t = const.tile([DEMO, 1], F32, tag="b2c", name="b2c")
    nc.sync.dma_start(b2_t[:], b2[:, None])
    b3_t = const.tile([P, 2], F32, tag="b3c", name="b3c")
    nc.sync.dma_start(b3_t[:], b3.rearrange("(i p) -> p i", p=P))
    for i in range(4):
        nc.sync.dma_start(W1_t[i][:], W1[:, i * P:(i + 1) * P])
        nc.sync.dma_start(W2_t[i][:], W2[i * P:(i + 1) * P, :])
    for i in range(2):
        nc.sync.dma_start(W3_t[i][:], W3[:, i * P:(i + 1) * P])

    hT = [const.tile([P, BP], F32, tag=f"hT{i}", name=f"hT{i}") for i in range(4)]
    for i in range(4):
        ps = psum.tile([P, BP], F32, tag="tp", bufs=2, name="demo_ps")
        nc.tensor.matmul(ps[:], lhsT=W1_t[i][:], rhs=demoT[:], start=True, stop=True)
        nc.scalar.activation(hT[i][:], ps[:], AF.Relu, bias=b1_t[:, i:i + 1], scale=1.0)
    ps_y = psum.tile([DEMO, BP], F32, tag="psSuf", bufs=2, name="demo_y")
    for i in range(4):
        nc.tensor.matmul(ps_y[:], lhsT=W2_t[i][:], rhs=hT[i][:],
                         start=(i == 0), stop=(i == 3))
    yT = const.tile([DEMO, BP], F32, tag="yT", name="yT")
    nc.scalar.activation(yT[:], ps_y[:], AF.Identity, bias=b2_t[:, 0:1], scale=1.0)
    nc.vector.tensor_add(yT[:], yT[:], demoT[:])
    deT = [const.tile([P, BP], F32, tag=f"deT{i}", name=f"deT{i}") for i in range(2)]
    for i in range(2):
        ps = psum.tile([P, BP], F32, tag="tp", bufs=2, name="demo_ps2")
        nc.tensor.matmul(ps[:], lhsT=W3_t[i][:], rhs=yT[:], start=True, stop=True)
        nc.scalar.activation(deT[i][:], ps[:], AF.Identity, bias=b3_t[:, i:i + 1], scale=1.0)
    de_t = []
    for i in range(2):
        ps = psum.tile([BP, P], F32, tag="tp", bufs=2, name="demo_tp")
        nc.tensor.transpose(ps[:], deT[i][:], id_t[:])
        de = const.tile([BP, P], F32, tag=f"de{i}", name=f"de{i}")
        nc.vector.tensor_copy(de[:], ps[:])
        de_t.append(de)
    for i in range(2):
        nc.sync.dma_start(
            out[0:OUT_ROWS, :].rearrange("(b n) m -> b n m", b=BP)[:, 0, i * P:(i + 1) * P],
            de_t[i][:])

    # ---------- main pipeline over the 7 event chunks ----------
    lnAf = big.tile([P, NCOL * MEM], R32, tag="lnAf", name="lnAf")
    contrib = big.tile([P, NCOL * MEM], R32, tag="contrib", name="contrib")
    eAll_t = big.tile([P, NCOL * MEM], F32, tag="eAll_t", name="eAll_t")
    Al_t = big.tile([P, NCH * MEM], F32, tag="Al_t", name="Al_t")
    Bf = big.tile([P, NCH * MEM], F32, tag="Bf", name="Bf")
    G = big.tile([P, NCOL * NCE], R32, tag="G", name="G")
    Gd = big.tile([P, NCOL * P], R32, tag="Gd", name="Gd")
    Gd2 = big.tile([P, NCOL * P], R32, tag="Gd2", name="Gd2")

    def cc(c, w):
        return slice(c * w, (c + 1) * w)

    def gblk(J, E_):
        return G[:, J * NCE + E_ * P: J * NCE + E_ * P + P]

    def emit_collision_logexp():
        # ln + suffix/total log-composition for the collision region.
        # SufLog (j>e) accumulates in psS; after exp'ing it, the prefix (j<=e)
        # keeps accumulating into the same bank so it becomes AllLog for free.
        nc.scalar.activation(lnAf[:], Al_t[:, 0:NCOL * MEM], AF.Ln)
        for E_ in range(NCOL):
            ps = psum.tile([P, MEM], F32, tag=("psSuf" if E_ % 2 else "tp"),
                           bufs=2, name="psSuf")
            js = list(range(E_, NCOL))
            for n_, J in enumerate(js):
                lhsT = Gd[:, cc(E_, P)] if J == E_ else gblk(J, E_)
                nc.tensor.matmul(ps[:], lhsT=lhsT, rhs=lnAf[:, cc(J, MEM)],
                                 start=(n_ == 0), stop=(n_ == len(js) - 1))
            eSuf = work.tile([P, MEM], F32, tag="eSuf", name="eSuf")
            nc.scalar.activation(eSuf[:], ps[:], AF.Exp)
            nc.gpsimd.tensor_tensor(contrib[:, cc(E_, MEM)], Bf[:, cc(E_, MEM)],
                                    eSuf[:], op=OP.mult)
            for J in range(0, E_ + 1):
                lhsT = Gd2[:, cc(E_, P)] if J == E_ else gblk(J, E_)
                nc.tensor.matmul(ps[:], lhsT=lhsT, rhs=lnAf[:, cc(J, MEM)],
                                 start=False, stop=(J == E_), skip_group_check=True)
            nc.scalar.activation(eAll_t[:, cc(E_, MEM)], ps[:], AF.Exp)

    def emit_collision_rows():
        for E_ in range(NCOL):
            psB = psum.tile([P, MEM], F32, tag="psB", bufs=2, name="psB")
            for J in range(NCOL):
                nc.tensor.matmul(psB[:], lhsT=gblk(J, E_),
                                 rhs=contrib[:, cc(J, MEM)],
                                 start=(J == 0), stop=(J == NCOL - 1))
            row = work.tile([P, MEM], F32, tag="row", name="row")
            nc.gpsimd.tensor_tensor(row[:], eAll_t[:, cc(E_, MEM)], init128[:],
                                    op=OP.mult)
            nc.vector.tensor_add(row[:], row[:], psB[:])
            nc.gpsimd.indirect_dma_start(
                out=out[:], out_offset=IndirectOffsetOnAxis(ap=ids_t[:, E_:E_ + 1], axis=0),
                in_=row[:], in_offset=None)

    th_t = big.tile([P, NCH * MEM], F32, tag="th_t", name="th_t")
    A_t = big.tile([P, NCH * MEM], F32, tag="A_t", name="A_t")
    init2 = const.tile([P, 2 * MEM], F32, tag="init2", name="init2")
    nc.vector.tensor_copy(init2[:, 0:MEM], init128[:])
    nc.vector.tensor_copy(init2[:, MEM:2 * MEM], init128[:])

    for c in range(NCH):
        # E/A gate matmuls (event-major out), fused into one [128,512] psum
        psEA = psum.tile([P, 2 * MEM], F32, tag="psEA", bufs=2, name="psEA")
        nc.tensor.matmul(psEA[:], lhsT=ones_row[:], rhs=beba_row[:],
                         start=True, stop=False)
        for i in range(2):
            nc.tensor.matmul(psEA[:], lhsT=xT[i][:, cc(c, P)], rhs=WeWa_t[i][:],
                             start=False, stop=(i == 1))
        nc.scalar.activation(th_t[:, cc(c, MEM)], psEA[:, 0:MEM], AF.Tanh, scale=0.5)
        nc.scalar.activation(A_t[:, cc(c, MEM)], psEA[:, MEM:2 * MEM], AF.Tanh)

    # D-level chain composition over chunk PAIRS (validity-free constants make
    # the scale columns chunk-independent).  Phase 1 launches every pair's
    # MkMs builds + partition-shift DMAs so the shift latency overlaps; phase 2
    # consumes them with the multiply/accumulate chains.
    PAIRS = [(0, 2), (2, 2), (4, 2), (6, 1)]
    MkMs_p, sh_p = {}, {}
    for pi, (c0, w) in enumerate(PAIRS):
        W = w * MEM
        sl = slice(c0 * MEM, c0 * MEM + W)
        th = th_t[:, sl]
        A = A_t[:, sl]
        MkMs = [work.tile([P, 2 * W], BF16, tag=f"MkMs{k}", name=f"MkMs{k}")
                for k in range(3)]
        sh = [work.tile([P, 2 * W], BF16, tag=f"sh{k}", name=f"sh{k}")
              for k in range(3)]
        MkMs_p[pi], sh_p[pi] = MkMs, sh
        for k in range(3):
            nc.vector.tensor_scalar(MkMs[k][:, 0:W], th, cv[:, k:k + 1],
                                    cv[:, 6 + k:7 + k], op0=OP.mult, op1=OP.add)
            nc.vector.tensor_scalar_mul(MkMs[k][:, W:2 * W], A, cv[:, 3 + k:4 + k])
            eng = (nc.sync, nc.scalar, nc.gpsimd)[(pi * 3 + k) % 3]
            eng.dma_start(sh[k][0:P - 1 - k, :], MkMs[k][1 + k:P, :])
        nc.vector.tensor_scalar(Al_t[:, sl], th, -0.5, 0.5, op0=OP.mult, op1=OP.add)
        nc.vector.tensor_copy(Bf[:, sl], A)
        if c0 == 0:
            for c in range(NCOL):
                # compare matrix rows for this j-chunk (ids are slot-unique)
                nc.vector.tensor_tensor(G[:, cc(c, NCE)],
                                        idsf_t[:, c:c + 1].to_broadcast([P, NCE]),
                                        ids_row128[:], op=OP.is_equal)
                nc.vector.tensor_mul(Gd[:, cc(c, P)],
                                     G[:, c * NCE + c * P: c * NCE + c * P + P], tri_t[:])
                nc.vector.tensor_tensor(Gd2[:, cc(c, P)],
                                        G[:, c * NCE + c * P: c * NCE + c * P + P],
                                        Gd[:, cc(c, P)], op=OP.subtract)
    for pi, (c0, w) in enumerate(PAIRS):
        W = w * MEM
        sl = slice(c0 * MEM, c0 * MEM + W)
        sh = sh_p[pi]
        Al = Al_t[:, sl]
        Bc = Bf[:, sl]
        # A chain (vector) and B chain (gpsimd)
        nc.vector.tensor_mul(Al[0:P - 1], Al[0:P - 1], sh[0][0:P - 1, 0:W])
        nc.vector.tensor_mul(Al[0:P - 2], Al[0:P - 2], sh[1][0:P - 2, 0:W])
        nc.vector.tensor_mul(Al[0:P - 3], Al[0:P - 3], sh[2][0:P - 3, 0:W])
        nc.gpsimd.tensor_tensor(Bc[0:P - 1], Bc[0:P - 1], sh[0][0:P - 1, 0:W], op=OP.mult)
        nc.gpsimd.tensor_tensor(Bc[0:P - 1], Bc[0:P - 1], sh[0][0:P - 1, W:2 * W], op=OP.add)
        nc.gpsimd.tensor_tensor(Bc[0:P - 2], Bc[0:P - 2], sh[1][0:P - 2, 0:W], op=OP.mult)
        nc.gpsimd.tensor_tensor(Bc[0:P - 2], Bc[0:P - 2], sh[1][0:P - 2, W:2 * W], op=OP.add)
        nc.gpsimd.tensor_tensor(Bc[0:P - 3], Bc[0:P - 3], sh[2][0:P - 3, 0:W], op=OP.mult)
        nc.gpsimd.tensor_tensor(Bc[0:P - 3], Bc[0:P - 3], sh[2][0:P - 3, W:2 * W], op=OP.add)
        if c0 == 0:
            nc.vector.tensor_scalar_max(Al, Al, 1e-30)
            emit_collision_logexp()
        else:
            # single-hit chunks: row = init*Al + Bf, scattered immediately
            rowd = work.tile([P, W], F32, tag="rowd", name="rowd")
            nc.gpsimd.tensor_tensor(rowd[:], Al, init2[:, 0:W], op=OP.mult)
            nc.vector.tensor_add(rowd[:], rowd[:], Bc)
            for j in range(w):
                nc.gpsimd.indirect_dma_start(
                    out=out[:],
                    out_offset=IndirectOffsetOnAxis(ap=ids_t[:, c0 + j:c0 + j + 1], axis=0),
                    in_=rowd[:, j * MEM:(j + 1) * MEM], in_offset=None)
        if pi == 2:
            emit_collision_rows()


def _assign_patients(gvalid):
    """Balanced 4-patients-per-core assignment by valid-group count (LPT)."""
    counts = gvalid.reshape(B, -1).sum(1)
    order = np.argsort(-counts, kind="stable")
    loads = [0] * N_CORES
    members = [[] for _ in range(N_CORES)]
    for p in order:
        c = min((c for c in range(N_CORES) if len(members[c]) < BP),
                key=lambda c: loads[c])
        members[c].append(int(p))
        loads[c] += int(counts[p])
    assert max(loads) * D <= S_C, f"core load {max(loads)} groups > {S_C // D}"
    return members


def _host_prep(inputs):
    """Index-only host prep: compaction, balancing, index tensors."""
    x = np.ascontiguousarray(np.asarray(inputs["input"], np.float32)).reshape(B, T * MOD * D, WD)
    mask = np.asarray(inputs["mask"])
    valid_mod = np.asarray(inputs["valid_mod"])
    node_ids = np.asarray(inputs["node_ids"])
    demo = np.ascontiguousarray(np.asarray(inputs["demo"], np.float32))

    dpat = np.arange(128) % 4
    cvecs = np.zeros((128, 12), np.float32)
    for k in (1, 2, 3):
        m = (dpat >= k).astype(np.float32)
        cvecs[:, k - 1] = -(2.0 ** -k) / 2.0 * m     # A_k: -ck/2 (tanh half-angle)
        cvecs[:, 2 + k] = (2.0 ** -k) * m            # S_k: ck
        cvecs[:, 5 + k] = 1.0 + cvecs[:, k - 1]      # B_k: 1 + A_k
    tri = np.tril(np.ones((128, 128), np.float32), -1)
    ident = np.eye(128, dtype=np.float32)

    gvalid = (mask[:, :, None] > 0) & (valid_mod > 0)   # [B, T, MOD]
    members = _assign_patients(gvalid)

    weights = {k: np.asarray(inputs[k], np.float32)
               for k in ("W1", "b1", "W2", "b2", "W3", "b3", "We", "be", "Wa", "ba",
                         "init_mem")}
    in_maps = []
    for core in range(N_CORES):
        pats = members[core]
        xg = np.zeros((S_C,), np.int32)
        idsv = np.empty((S_C,), np.int32)
        idsv[:] = OUT_ROWS + (np.arange(S_C) % PAD_ROWS)  # pads -> scratch rows
        vf = np.zeros((S_C,), np.float32)
        # groups containing any multi-hit-slot event go first (chunks 0..NCOL-1,
        # the only region the G composition covers); per patient in time order.
        col_list, norm_list = [], []
        for slot, b in enumerate(pats):
            tms = np.nonzero(gvalid[b].reshape(T * MOD))[0]
            idsm = node_ids[b].reshape(T * MOD, D)
            uniq, cnt = np.unique(idsm[tms].reshape(-1), return_counts=True)
            multi = set(uniq[cnt >= 2].tolist())
            for tm in tms:
                dst = col_list if any(int(v) in multi for v in idsm[tm]) else norm_list
                dst.append((slot, int(tm)))
        assert len(col_list) * D <= NCE, \
            f"core {core}: {len(col_list)} collision groups > {NCE // D}"
        e = 0
        for slot, tm in col_list + norm_list:
            b = pats[slot]
            for d in range(D):
                xg[e] = slot * (T * MOD * D) + tm * D + d
                idsv[e] = slot * N_NODES + int(
                    node_ids[b, tm // MOD, tm % MOD, d])
                vf[e] = 1.0
                e += 1
        xe = x[pats].reshape(X_ROWS, WD)[xg].T     # [WD, S_C]
        in_maps.append({
            "xT0": np.ascontiguousarray(xe[0:128]),
            "xT1": np.ascontiguousarray(xe[128:256]),
            "ids": idsv, "validf": vf,
            "cvecs": cvecs, "tri": tri, "ident": ident,
            "demo": np.ascontiguousarray(demo[pats]),
            **weights,
        })
    return in_maps, members


def get_nc():
    if "nc" not in _NC_CACHE:
        _NC_CACHE["nc"] = _build_nc()
    return _NC_CACHE["nc"]


def kernel(**inputs) -> np.ndarray:
    nc = get_nc()
    in_maps, members = _host_prep(inputs)
    res = bass_utils.run_bass_kernel_spmd(nc, in_maps, core_ids=list(range(N_CORES)))
    out = np.empty((B, N_NODES, MEM), np.float32)
    for core in range(N_CORES):
        block = res.results[core]["out"][:OUT_ROWS].reshape(BP, N_NODES, MEM)
        for slot, b in enumerate(members[core]):
            out[b] = block[slot]
    return out


if __name__ == "__main__":
    ref = {}
    exec(open("/root/problem/reference.py").read(), ref)
    inputs = {k: np.asarray(v) for k, v in ref["setup_inputs"]().items()}
    got = kernel(**inputs)
    want = np.asarray(ref["reference"](**inputs))
    err = np.abs(got - want).max() / np.abs(want).max()
    print("rel err:", err)

